# revision 18
# baseline (speedup 1.0000x reference)
"""Trainium2 Bass kernel for causal self-attention with RoPE.

Sharding: 8 cores = 2 batches x 4 head-groups (4 heads each).
Each core computes its batch's qkv projection for its heads, RoPE,
causal flash-attention, and a partial output projection; the host sums
the 4 partials per batch.

All matmuls run as fp32r (full-rate) except scores, whose operands are
bf16 (produced by the RoPE pass). Softmax uses no max-subtraction
(scores are O(5) bounded), and the denominator comes from an extra
ones-column in the PV stationary operand.
"""

import os

import numpy as np

NUM_HEADS = 16
B, T, C = 2, 2048, 1024
D = C // NUM_HEADS  # 64
HPC = 4             # heads per core
NCORES = 8

_CACHE = {}

LAST_EXEC_NS = None
LAST_RESULTS = None


def _build_body(nc):
    import concourse.bass as bass
    import concourse.mybir as mybir
    import concourse.tile as tile
    from contextlib import ExitStack

    F32 = mybir.dt.float32
    F32R = mybir.dt.float32r
    BF16 = mybir.dt.bfloat16
    AF = mybir.ActivationFunctionType

    xT = nc.dram_tensor("xT", [C, T], F32R, kind="ExternalInput").ap()
    wT = nc.dram_tensor("wT", [C, 768], F32R, kind="ExternalInput").ap()
    projT = nc.dram_tensor("projT", [256, C], BF16, kind="ExternalInput").ap()
    CS = nc.dram_tensor("CS", [128, T], F32, kind="ExternalInput").ap()
    SN = nc.dram_tensor("SN", [128, T], F32, kind="ExternalInput").ap()
    maskc = nc.dram_tensor("maskc", [128, 128], F32, kind="ExternalInput").ap()
    ident = nc.dram_tensor("ident", [128, 128], F32, kind="ExternalInput").ap()
    out = nc.dram_tensor("out", [T, C], F32, kind="ExternalOutput").ap()
    linv_dram = nc.dram_tensor("linv_scratch", [64, 128], F32).ap()

    with tile.TileContext(nc) as tc, ExitStack() as ctx:
        singles = ctx.enter_context(tc.tile_pool(name="singles", bufs=1))
        stream = ctx.enter_context(tc.tile_pool(name="stream", bufs=3))
        ptiles = ctx.enter_context(tc.tile_pool(name="ptiles", bufs=3))

        cs_sb = singles.tile([128, T], F32)
        nc.sync.dma_start(out=cs_sb[:], in_=CS)
        sn_sb = singles.tile([128, T], F32)
        nc.sync.dma_start(out=sn_sb[:], in_=SN)
        mask_sb = singles.tile([128, 128], F32)
        nc.sync.dma_start(out=mask_sb[:], in_=maskc)
        id_sb = singles.tile([128, 128], F32)
        nc.sync.dma_start(out=id_sb[:], in_=ident)
        w_sb = singles.tile([128, 8, 768], F32R)
        for ci in range(8):
            nc.sync.dma_start(
                out=w_sb[:, ci, :], in_=wT[ci * 128:(ci + 1) * 128, :]
            )
        pj_sb = singles.tile([128, 2, C], BF16)
        for hpi in range(2):
            nc.sync.dma_start(
                out=pj_sb[:, hpi, :], in_=projT[hpi * 128:(hpi + 1) * 128, :]
            )

        q_rot = singles.tile([128, 2, T], BF16)
        k_rot = singles.tile([128, 2, T], BF16)
        v_sb = singles.tile([128, 16, 65 * HPC], BF16)
        # per-(head, q-block) attention outputs at partitions 0-64
        # (row 64 = softmax denominator); block index r = h*4 + qb
        u_sb = singles.tile([65, 16, 512], F32)
        u2_sb = singles.tile([128, 2, T], BF16)
        l_sb = singles.tile([16, 512], F32)
        linv_col = singles.tile([128, 64], F32)
        linv_rows = singles.tile([64, 128], F32)

        # ones columns interleaved with v (col 64 of every 65-col head block)
        v_h = v_sb[:].rearrange("p t (h c) -> p t h c", c=65)
        nc.vector.memset(v_h[:, :, :, 64:65], 1.0)

        # ---- Phase 1: QKV projection + RoPE ----
        with tc.tile_pool(name="qkvps", bufs=1, space="PSUM") as qkvps:
            for ch in range(4):
                tok = slice(ch * 512, (ch + 1) * 512)
                q_ps = [qkvps.tile([128, 512], F32, tag=f"qps{ft}", name=f"qps{ft}") for ft in range(2)]
                k_ps = [qkvps.tile([128, 512], F32, tag=f"kps{ft}", name=f"kps{ft}") for ft in range(2)]
                v_ps = [qkvps.tile([128, 512], F32, tag=f"vps{hb}", name=f"vps{hb}") for hb in range(2)]
                for ci in range(8):
                    xt = stream.tile([128, 512], F32R, tag="xt")
                    nc.sync.dma_start(out=xt[:], in_=xT[ci * 128:(ci + 1) * 128, tok])
                    st = ci == 0
                    sp = ci == 7
                    for ft in range(2):
                        nc.tensor.matmul(
                            q_ps[ft][:],
                            w_sb[:, ci, ft * 128:(ft + 1) * 128],
                            xt[:],
                            start=st, stop=sp,
                        )
                        nc.tensor.matmul(
                            k_ps[ft][:],
                            w_sb[:, ci, 256 + ft * 128:256 + (ft + 1) * 128],
                            xt[:],
                            start=st, stop=sp,
                        )
                    for ts in range(4):
                        nc.tensor.matmul(
                            v_ps[ts // 2][:, (ts % 2) * 256:(ts % 2 + 1) * 256],
                            xt[:, ts * 128:(ts + 1) * 128],
                            w_sb[:, ci, 512:768],
                            start=(st and ts % 2 == 0), stop=(sp and ts % 2 == 1),
                        )
                # RoPE: dest[e] = ps[e]*c + ps[o]*(-s); dest[o] = ps[o]*c + ps[e]*s
                # CS = [c,c,c,c]; SN = [+s, -s, +s, -s] per 32-block. The
                # partition swap (e<->o 32-blocks) rides on a SBUF->SBUF DMA
                # since compute engines cannot cross partitions.
                for src_ps, dst in ((q_ps, q_rot), (k_ps, k_rot)):
                    for ft in range(2):
                        t1 = stream.tile([128, 512], BF16, tag="t1")
                        t2 = stream.tile([128, 512], BF16, tag="t2")
                        t2s = stream.tile([128, 512], BF16, tag="t2s")
                        nc.vector.tensor_mul(t1[:], src_ps[ft][:], cs_sb[:, tok])
                        nc.vector.tensor_mul(t2[:], src_ps[ft][:], sn_sb[:, tok])
                        for hb in range(2):
                            e = slice(hb * 64, hb * 64 + 32)
                            o = slice(hb * 64 + 32, hb * 64 + 64)
                            nc.sync.dma_start(out=t2s[e, :], in_=t2[o, :])
                            nc.sync.dma_start(out=t2s[o, :], in_=t2[e, :])
                        nc.vector.tensor_add(dst[:, ft, tok], t1[:], t2s[:])
                for ts in range(4):
                    tokt = ch * 4 + ts
                    src = v_ps[ts // 2][:, (ts % 2) * 256:(ts % 2 + 1) * 256]
                    nc.vector.tensor_copy(
                        v_h[:, tokt, :, 0:64],
                        src.rearrange("p (h c) -> p h c", h=4),
                    )

        # ---- Phase 2: causal attention (S^T layout) ----
        with tc.tile_pool(name="attnps", bufs=2, space="PSUM") as attnps:
            for hp in range(2):
                for qb in range(4):
                    nkt = 4 * qb + 4
                    uA = attnps.tile([65, 512], F32, tag="uA")
                    uB = attnps.tile([65, 512], F32, tag="uB")
                    hA = 2 * hp
                    hB = 2 * hp + 1
                    for kt in range(nkt):
                        j = kt - 4 * qb
                        off = max(j, 0) * 128
                        ks = slice(kt * 128, (kt + 1) * 128)
                        qs = slice(qb * 512 + off, (qb + 1) * 512)
                        sA = attnps.tile([128, 512], F32, tag="sA")
                        sB = attnps.tile([128, 512], F32, tag="sB")
                        nc.tensor.matmul(
                            sA[:, off:512], k_rot[0:64, hp, ks], q_rot[0:64, hp, qs],
                            start=True, stop=True,
                        )
                        nc.tensor.matmul(
                            sB[:, off:512], k_rot[64:128, hp, ks], q_rot[64:128, hp, qs],
                            start=True, stop=True,
                        )
                        if j >= 0:
                            nc.vector.tensor_add(
                                sA[:, off:off + 128], sA[:, off:off + 128], mask_sb[:]
                            )
                            nc.vector.tensor_add(
                                sB[:, off:off + 128], sB[:, off:off + 128], mask_sb[:]
                            )
                        pA = ptiles.tile([128, 512], BF16, tag="pA")
                        pB = ptiles.tile([128, 512], BF16, tag="pB")
                        nc.scalar.activation(pA[:, off:512], sA[:, off:512], AF.Exp)
                        nc.scalar.activation(pB[:, off:512], sB[:, off:512], AF.Exp)
                        nc.tensor.matmul(
                            uA[0:65, off:512],
                            v_sb[:, kt, hA * 65:(hA + 1) * 65],
                            pA[:, off:512],
                            start=(kt == 0), stop=(kt == nkt - 1),
                        )
                        nc.tensor.matmul(
                            uB[0:65, off:512],
                            v_sb[:, kt, hB * 65:(hB + 1) * 65],
                            pB[:, off:512],
                            start=(kt == 0), stop=(kt == nkt - 1),
                        )
                    for u_ps, h in ((uA, hA), (uB, hB)):
                        r = h * 4 + qb
                        nc.vector.tensor_copy(u_sb[:, r, :], u_ps[0:65, :])
                        nc.sync.dma_start(out=l_sb[r:r + 1, :], in_=u_sb[64:65, r, :])

        # ---- Phase 3: softmax denominators -> per-column reciprocals ----
        import concourse.bass as bass_mod  # for raw AP construction

        with tc.tile_pool(name="miscps", bufs=1, space="PSUM") as miscps:
            lt_ps = miscps.tile([128, 64], F32, tag="lt")
            for s in range(4):
                nc.tensor.matmul(
                    lt_ps[:, s * 16:(s + 1) * 16],
                    l_sb[0:16, s * 128:(s + 1) * 128],
                    id_sb[0:16, 0:16],
                    start=True, stop=True, is_transpose=True,
                )
            nc.vector.reciprocal(linv_col[:], lt_ps[:])
            lvt_ps = miscps.tile([64, 128], F32, tag="lvt")
            nc.tensor.matmul(
                lvt_ps[:], linv_col[:], id_sb[:], start=True, stop=True,
                is_transpose=True,
            )
            nc.vector.tensor_copy(linv_rows[:], lvt_ps[:])
        nc.sync.dma_start(out=linv_dram, in_=linv_rows[:])
        # normalize each block and DMA it into the 128-partition proj
        # operand (heads 2hp at parts 0-63, 2hp+1 at parts 64-127)
        for h in range(4):
            hp, hh = h // 2, h % 2
            for qb in range(4):
                r = h * 4 + qb
                # linv_dram row j = s*16 + r holds 1/l for block r, segment s
                src = bass_mod.AP(
                    linv_dram.tensor, r * 128, [[0, 64], [2048, 4], [1, 128]]
                )
                lb = stream.tile([64, 512], F32, tag="lb", name="lb")
                nc.sync.dma_start(out=lb[:], in_=src)
                u2t = stream.tile([64, 512], BF16, tag="u2t", name="u2t")
                nc.vector.tensor_mul(u2t[:], u_sb[0:64, r, :], lb[:])
                nc.sync.dma_start(
                    out=u2_sb[hh * 64:(hh + 1) * 64, hp, qb * 512:(qb + 1) * 512],
                    in_=u2t[:],
                )

        # ---- Phase 4: output projection (partial; host sums across cores) ----
        with tc.tile_pool(name="projps", bufs=2, space="PSUM") as projps:
            for m in range(16):
                ms = slice(m * 128, (m + 1) * 128)
                for nh in range(2):
                    pp = projps.tile([128, 512], F32, tag="pp")
                    nc.tensor.matmul(
                        pp[:],
                        u2_sb[:, 0, ms],
                        pj_sb[:, 0, nh * 512:(nh + 1) * 512],
                        start=True, stop=False,
                    )
                    nc.tensor.matmul(
                        pp[:],
                        u2_sb[:, 1, ms],
                        pj_sb[:, 1, nh * 512:(nh + 1) * 512],
                        start=False, stop=True,
                    )
                    ob = stream.tile([128, 512], F32, tag="ob")
                    nc.any.tensor_copy(ob[:], pp[:])
                    nc.sync.dma_start(out=out[ms, nh * 512:(nh + 1) * 512], in_=ob[:])
    return nc


def _get_nc():
    if "nc" not in _CACHE:
        import concourse.bacc as bacc

        nc = bacc.Bacc("TRN2", target_bir_lowering=False, debug=False)
        _build_body(nc)
        nc.compile()
        _CACHE["nc"] = nc
    return _CACHE["nc"]


def _prep_in_maps(x, freqs_cos, freqs_sin, qkv_w, proj_w):
    x = np.asarray(x, dtype=np.float32)
    cos = np.asarray(freqs_cos, dtype=np.float32)
    sin = np.asarray(freqs_sin, dtype=np.float32)
    qkv_w = np.asarray(qkv_w, dtype=np.float32)
    proj_w = np.asarray(proj_w, dtype=np.float32)

    sq = np.float32((1.0 / np.sqrt(D)) ** 0.5)
    cosT = np.ascontiguousarray(cos.T) * sq  # (32, T)
    sinT = np.ascontiguousarray(sin.T) * sq
    CS = np.tile(cosT, (4, 1)).astype(np.float32)
    SN = np.tile(np.concatenate([sinT, -sinT], axis=0), (2, 1)).astype(np.float32)
    f = np.arange(128)
    maskc = np.where(f[None, :] >= f[:, None], 0.0, -1e30).astype(np.float32)
    ident = np.eye(128, dtype=np.float32)
    perm = np.concatenate([np.arange(0, D, 2), np.arange(1, D, 2)])

    in_maps = []
    for core in range(NCORES):
        b = core // 4
        g = core % 4
        heads = [4 * g + j for j in range(HPC)]
        q_rows = np.concatenate([h * D + perm for h in heads])
        k_rows = np.concatenate([C + h * D + perm for h in heads])
        v_rows = np.concatenate([2 * C + h * D + np.arange(D) for h in heads])
        wTc = np.ascontiguousarray(
            np.concatenate(
                [qkv_w[q_rows, :], qkv_w[k_rows, :], qkv_w[v_rows, :]], axis=0
            ).T
        )  # (1024, 768)
        vcols = np.concatenate([h * D + np.arange(D) for h in heads])
        import ml_dtypes

        projTc = np.ascontiguousarray(proj_w[:, vcols].T).astype(ml_dtypes.bfloat16)
        xTc = np.ascontiguousarray(x[b].T)  # (1024, 2048)
        in_maps.append(
            {
                "xT": xTc,
                "wT": wTc,
                "projT": projTc,
                "CS": CS,
                "SN": SN,
                "maskc": maskc,
                "ident": ident,
            }
        )
    return in_maps


def _get_runner():
    """Build (once) a jitted 8-core shard_map executable mirroring
    bass2jax.run_bass_via_pjrt, without donation so it can be re-run for
    timing with device-resident inputs."""
    if "runner" in _CACHE:
        return _CACHE["runner"]
    import jax
    import concourse.mybir as mybir
    from concourse import bass2jax
    from jax.experimental.shard_map import shard_map
    from jax.sharding import Mesh, PartitionSpec

    nc = _get_nc()
    bass2jax.install_neuronx_cc_hook()

    in_names = []
    out_names = []
    out_avals = []
    zero_outs = []
    pname = nc.partition_id_tensor.name if nc.partition_id_tensor else None
    for alloc in nc.m.functions[0].allocations:
        if not isinstance(alloc, mybir.MemoryLocationSet):
            continue
        name = alloc.memorylocations[0].name
        if alloc.kind == "ExternalInput":
            if name != pname:
                in_names.append(name)
        elif alloc.kind == "ExternalOutput":
            shape = tuple(alloc.tensor_shape)
            dtype = mybir.dt.np(alloc.dtype)
            out_names.append(name)
            out_avals.append(jax.core.ShapedArray(shape, dtype))
            zero_outs.append(np.zeros(shape, dtype))
    n_params = len(in_names)
    all_names = list(in_names) + list(out_names)
    if pname is not None:
        all_names.append(pname)

    def _body(*args):
        operands = list(args)
        if pname is not None:
            operands.append(bass2jax.partition_id_tensor())
        outs = bass2jax._bass_exec_p.bind(
            *operands,
            out_avals=tuple(out_avals),
            in_names=tuple(all_names),
            out_names=tuple(out_names),
            lowering_input_output_aliases=(),
            sim_require_finite=True,
            sim_require_nnan=True,
            nc=nc,
        )
        return tuple(outs)

    devices = jax.devices()[:NCORES]
    mesh = Mesh(np.asarray(devices), ("core",))
    nin = n_params + len(out_names)
    sharded = jax.jit(
        shard_map(
            _body,
            mesh=mesh,
            in_specs=(PartitionSpec("core"),) * nin,
            out_specs=(PartitionSpec("core"),) * len(out_names),
            check_rep=False,
        ),
        keep_unused=True,
    )
    _CACHE["runner"] = (sharded, in_names, out_names, zero_outs, mesh)
    return _CACHE["runner"]


def kernel(x, freqs_cos, freqs_sin, qkv_w, proj_w):
    import jax
    from jax.sharding import NamedSharding, PartitionSpec

    global LAST_EXEC_NS, LAST_RESULTS
    sharded, in_names, out_names, zero_outs, mesh = _get_runner()
    in_maps = _prep_in_maps(x, freqs_cos, freqs_sin, qkv_w, proj_w)

    concat_in = [
        np.concatenate([in_maps[c][n] for c in range(NCORES)], axis=0)
        for n in in_names
    ]
    concat_zero = [
        np.zeros((NCORES * z.shape[0], *z.shape[1:]), z.dtype) for z in zero_outs
    ]
    sharding = NamedSharding(mesh, PartitionSpec("core"))
    dev_args = [jax.device_put(a, sharding) for a in concat_in + concat_zero]

    out_arrs = sharded(*dev_args)
    jax.block_until_ready(out_arrs)

    iters = int(os.environ.get("KERNEL_TIME_ITERS", "0"))
    if iters > 0:
        import time

        t0 = time.monotonic()
        for _ in range(iters):
            out_arrs = sharded(*dev_args)
        jax.block_until_ready(out_arrs)
        t1 = time.monotonic()
        LAST_EXEC_NS = (t1 - t0) / iters * 1e9

    out = np.asarray(out_arrs[out_names.index("out")]).reshape(NCORES, T, C)
    return np.stack(
        [
            out[0] + out[1] + out[2] + out[3],
            out[4] + out[5] + out[6] + out[7],
        ],
        axis=0,
    )
